# revision 1
# baseline (speedup 1.0000x reference)
"""DiagWinAttention TRN2 kernel.

Strategy (per sharding hint): pure data-parallel over the leading window
dimension nw=8192 -> 1024 windows per NeuronCore (8 cores). The bias table,
LayerNorm and projection params are replicated; the mask is tiled mod 128 so
it is replicated too. Each core runs the fused window-attention block
(QK^T + rel-pos bias + diag mask, softmax, AV, residual, LayerNorm, proj)
on its shard; results are concatenated on the host.

Hardcoded problem shapes: q/k/v [8192, 64, 96], mask [128, 64, 64],
bias_table [225, 6], 6 heads x 16 ch, 8x8 windows (SH=SW=1).
"""

import numpy as np

WH, WW = 8, 8
NH = 6
ED = 96
CH = ED // NH
NP = WH * WW  # 64
L = NP        # SH=SW=1
SCALE = CH ** -0.5
EPS = 1e-5
NEG = -10.0 ** 9
N_CORES = 8


def _rel_index():
    coords = np.stack(np.meshgrid(np.arange(WH), np.arange(WW), indexing="ij"))
    cf = coords.reshape(2, -1)
    rel = cf[:, :, None] - cf[:, None, :]
    rel = np.moveaxis(rel, 0, -1).astype(np.int64)
    rel[..., 0] += WH - 1
    rel[..., 0] *= 2 * WW - 1
    rel[..., 1] += WW - 1
    return rel.sum(-1).reshape(-1)


def _np_forward(q_shard, k_shard, v_shard, add_bias, gamma, beta, w, b):
    """Single-shard forward in float32 numpy. add_bias: [128, NH, NP, NP]
    combined (rel-pos bias + effective mask) additive term, indexed w%128."""
    nw = q_shard.shape[0]
    qh = q_shard.reshape(nw, NP, NH, CH).transpose(0, 2, 1, 3)  # [nw,nh,np,ch]
    kh = k_shard.reshape(nw, NP, NH, CH).transpose(0, 2, 1, 3)
    vh = v_shard.reshape(nw, NP, NH, CH).transpose(0, 2, 1, 3)
    attn = np.einsum("wnqc,wnkc->wnqk", qh * SCALE, kh)
    m = add_bias[np.arange(nw) % add_bias.shape[0]]  # [nw,nh,np,np]
    attn = attn + m
    attn = attn - attn.max(axis=-1, keepdims=True)
    p = np.exp(attn)
    p = p / p.sum(axis=-1, keepdims=True)
    o = np.einsum("wnqk,wnkc->wnqc", p, vh)
    o = o.transpose(0, 2, 1, 3).reshape(nw, NP, ED)
    x = o + q_shard
    mu = x.mean(-1, keepdims=True)
    var = ((x - mu) ** 2).mean(-1, keepdims=True)
    x = (x - mu) / np.sqrt(var + EPS) * gamma + beta
    return x @ w.T + b


def kernel(query, key, value, mask, bias_table, norm_gamma, norm_beta,
           proj_w, proj_b, is_masked):
    query = np.asarray(query, np.float32)
    key_a = np.asarray(key, np.float32)
    value_a = np.asarray(value, np.float32)
    mask = np.asarray(mask, np.float32)
    bias_table = np.asarray(bias_table, np.float32)
    gamma = np.asarray(norm_gamma, np.float32)
    beta = np.asarray(norm_beta, np.float32)
    w = np.asarray(proj_w, np.float32)
    b = np.asarray(proj_b, np.float32)

    # Host prep of the replicated additive term: rel-pos bias + effective mask.
    rel = _rel_index()
    bias = bias_table[rel].reshape(NP, NP, NH).transpose(2, 0, 1)  # [nh,np,np]
    em = mask.copy()
    if int(np.asarray(is_masked)):
        di = np.arange(NP)
        em[:, di, di] = 1.0
    em = np.where(em != 0, NEG, em).astype(np.float32)  # [128,np,np]
    add_bias = bias[None] + em[:, None]  # [128, nh, np, np]

    nw = query.shape[0]
    per = nw // N_CORES

    q_out = None
    try:
        q_out = _run_on_neuron(query, key_a, value_a, add_bias, gamma, beta,
                               w, b, per)
    except Exception as e:  # pragma: no cover - hardware fallback
        import sys
        print(f"[kernel] neuron path failed ({type(e).__name__}: {e}); "
              f"falling back to host compute", file=sys.stderr)
    if q_out is None:
        shards = [
            _np_forward(query[i * per:(i + 1) * per],
                        key_a[i * per:(i + 1) * per],
                        value_a[i * per:(i + 1) * per],
                        add_bias, gamma, beta, w, b)
            for i in range(N_CORES)
        ]
        q_out = np.concatenate(shards, 0).astype(np.float32)

    # key/value pass through partition + inverse partition unchanged (SH=SW=1).
    return q_out, key_a, value_a


def _run_on_neuron(query, key_a, value_a, add_bias, gamma, beta, w, b, per):
    """Data-parallel execution on the 8 NeuronCores via jax/axon."""
    import jax
    import jax.numpy as jnp

    devs = jax.devices()
    if len(devs) < N_CORES:
        raise RuntimeError(f"need {N_CORES} devices, have {len(devs)}")
    devs = devs[:N_CORES]

    def fwd(q, k, v, ab, g, bt, pw, pb):
        nwq = q.shape[0]
        qh = q.reshape(nwq, NP, NH, CH).transpose(0, 2, 1, 3)
        kh = k.reshape(nwq, NP, NH, CH).transpose(0, 2, 1, 3)
        vh = v.reshape(nwq, NP, NH, CH).transpose(0, 2, 1, 3)
        attn = jnp.einsum("wnqc,wnkc->wnqk", qh * SCALE, kh)
        m = jnp.tile(ab, (nwq // ab.shape[0], 1, 1, 1))
        attn = attn + m
        p = jax.nn.softmax(attn, axis=-1)
        o = jnp.einsum("wnqk,wnkc->wnqc", p, vh)
        o = o.transpose(0, 2, 1, 3).reshape(nwq, NP, ED)
        x = o + q
        mu = jnp.mean(x, axis=-1, keepdims=True)
        var = jnp.mean(jnp.square(x - mu), axis=-1, keepdims=True)
        x = (x - mu) * jax.lax.rsqrt(var + EPS) * g + bt
        return x @ pw.T + pb

    jf = jax.jit(fwd)
    futs = []
    for i, d in enumerate(devs):
        sl = slice(i * per, (i + 1) * per)
        args = [jax.device_put(a, d) for a in
                (query[sl], key_a[sl], value_a[sl], add_bias, gamma, beta, w, b)]
        futs.append(jf(*args))
    shards = [np.asarray(f) for f in futs]
    return np.concatenate(shards, 0).astype(np.float32)

